# revision 4
# baseline (speedup 1.0000x reference)
"""DepAttention kernel for Trainium2 (Bass/Tile), data-parallel over batch.

score[b,i,j] = (<val[b,i],val[b,j]> + <dep[b,i,j],dep[b,j,i]>) / sqrt(D)
out = exp(score)*adj / (rowsum(exp(score)*adj) + 1e-10)

score is symmetric in (i,j), so each core (one batch element) computes
each unordered pair's dep dot product exactly once and mirrors the rest
with PE transposes. dep is sent in TWO fp16 copies chosen so every DMA
is contiguous 4KB-per-partition: depA[i,j,:] = dep[i,j,:] and
depB[i,j,:] = dep[j,i,:]; the product depA*depB followed by a sum over
d gives the dep score. fp16 halves both HBM traffic and DVE multiply
time (2x packed mode); the d-reduction runs as an in-place fp16 add
tree (128->64->32->16, each step 2x) and a final f32 reduce_sum over 16.

Work partition per 128-row block pair (N=256 = 2 blocks):
 - off-diag block (0,1): full 128x128, chunked by 16 cols; mirrored
   into (1,0) by transposing after the val part is added (val score is
   symmetric too).
 - diagonal blocks: strict-triangle column chunks packed EXACTLY into
   128 partitions: block00's upper chunk c (partitions 0..16c) rides
   with block11's lower chunk c-1 (partitions 16c..128) in one tile,
   c = 1..7. Both reduce straight into their score tiles at native
   partitions. The 16x16 diagonal mini-squares are computed full
   (both triangles at once) as one stacked [128,16,128] tile per block
   and scattered after the mirror-add, so no masks are needed: at
   mirror time the uncomputed regions are zero.
"""

import numpy as np

import concourse.bacc as bacc
import concourse.tile as tile
import concourse.mybir as mybir
from concourse.bass_utils import run_bass_kernel_spmd

B, N, D = 8, 256, 128
W = 16        # columns per chunk
NCH = 128 // W
SCALE = 1.0 / np.sqrt(np.float32(D))
EPS = 1e-10
F32 = mybir.dt.float32
F16 = mybir.dt.float16

_NC = None


def build_nc(reps=1):
    """reps>1 unrolls the whole computation (for paired-slope timing)."""
    nc = bacc.Bacc("TRN2", target_bir_lowering=False, debug=False, num_devices=8)

    depA = nc.dram_tensor("depA", [N, N, D], F16, kind="ExternalInput")
    depB = nc.dram_tensor("depB", [N, N, D], F16, kind="ExternalInput")
    valT = nc.dram_tensor("valT", [D, N], F32, kind="ExternalInput")
    adj = nc.dram_tensor("adj", [N, N], F32, kind="ExternalInput")
    ident = nc.dram_tensor("ident", [128, 128], F32, kind="ExternalInput")
    out = nc.dram_tensor("out", [N, N], F32, kind="ExternalOutput")

    with tile.TileContext(nc) as tc:
        with (
            tc.tile_pool(name="a", bufs=6) as a_pool,
            tc.tile_pool(name="b", bufs=6) as b_pool,
            tc.tile_pool(name="persist", bufs=1) as pp,
            tc.tile_pool(name="psum", bufs=1, space="PSUM") as psp,
        ):
            # persistent tiles
            vt = pp.tile([D, N], F32, tag="vt")
            id_t = pp.tile([128, 128], F32, tag="id")
            adj_t = [
                pp.tile([128, N], F32, tag=f"adj{i}", name=f"adj{i}") for i in range(2)
            ]
            scratch = pp.tile([128, 1], F32, tag="scratch")

            nc.gpsimd.dma_start(vt[:], valT[:])
            nc.gpsimd.dma_start(id_t[:], ident[:])
            for i in range(2):
                nc.gpsimd.dma_start(adj_t[i][:], adj[128 * i : 128 * (i + 1), :])
            # prime the ACT exp table before the epilogue needs it
            nc.vector.memset(scratch[:], 0.0)
            nc.scalar.activation(
                scratch[:], scratch[:], mybir.ActivationFunctionType.Exp, scale=1.0
            )

            def chain_and_reduce(a_t, b_t, outs):
                """In-place fp16 add tree over d then f32 reduces.
                outs = list of (out_ap, part_lo, part_hi)."""
                nc.vector.tensor_mul(a_t[:], a_t[:], b_t[:])
                nc.vector.tensor_add(
                    a_t[:, :, 0:64], a_t[:, :, 0:64], a_t[:, :, 64:128]
                )
                nc.vector.tensor_add(
                    a_t[:, :, 0:32], a_t[:, :, 0:32], a_t[:, :, 32:64]
                )
                nc.vector.tensor_add(
                    a_t[:, :, 0:16], a_t[:, :, 0:16], a_t[:, :, 16:32]
                )
                for out_ap, lo, hi in outs:
                    nc.vector.reduce_sum(
                        out_ap, a_t[lo:hi, :, 0:16], axis=mybir.AxisListType.X
                    )

            for _rep in range(reps):
                score = [
                    pp.tile([128, N], F32, tag=f"score{i}", name=f"score{i}", bufs=2)
                    for i in range(2)
                ]
                expv = [
                    pp.tile([128, N], F32, tag=f"expv{i}", name=f"expv{i}", bufs=2)
                    for i in range(2)
                ]
                mtmp = [
                    pp.tile([128, W], F32, tag=f"mtmp{i}", name=f"mtmp{i}", bufs=2)
                    for i in range(2)
                ]
                den = [
                    pp.tile([128, 1], F32, tag=f"den{i}", name=f"den{i}", bufs=2)
                    for i in range(2)
                ]
                denb = [
                    pp.tile([128, 1], F32, tag=f"denb{i}", name=f"denb{i}", bufs=2)
                    for i in range(2)
                ]
                rec = [
                    pp.tile([128, 1], F32, tag=f"rec{i}", name=f"rec{i}", bufs=2)
                    for i in range(2)
                ]
                psum_sv = [
                    psp.tile([128, N], F32, tag=f"sv{i}", name=f"sv{i}", bufs=1)
                    for i in range(2)
                ]

                # zero regions that the mirror-add will read where uncomputed
                nc.gpsimd.memset(score[0][:, 0:128], 0.0)
                nc.gpsimd.memset(score[1][:, 128:256], 0.0)

                # val part: score_val[I] = valT[:, I*128:+128].T @ valT
                for i in range(2):
                    nc.tensor.matmul(
                        psum_sv[i][:],
                        vt[:, 128 * i : 128 * (i + 1)],
                        vt[:],
                        start=True,
                        stop=True,
                    )

                # --- diagonal 16x16 mini-squares, stacked per block ---
                for blk in range(2):
                    base = 128 * blk
                    a_t = a_pool.tile([128, W, D], F16, name="a_t", tag="a_t")
                    b_t = b_pool.tile([128, W, D], F16, name="b_t", tag="b_t")
                    for m in range(NCH):
                        lo, hi = 16 * m, 16 * m + 16
                        nc.sync.dma_start(
                            a_t[lo:hi, :, :],
                            depA[base + lo : base + hi, base + lo : base + hi, :],
                        )
                        nc.scalar.dma_start(
                            b_t[lo:hi, :, :],
                            depB[base + lo : base + hi, base + lo : base + hi, :],
                        )
                    chain_and_reduce(a_t, b_t, [(mtmp[blk][:], 0, 128)])

                # --- diagonal strict-triangle pair chunks c=1..7 ---
                for c in range(1, NCH):
                    p = 16 * c
                    cols0 = (16 * c, 16 * c + 16)          # block00 upper cols
                    cols1 = (128 + 16 * (c - 1), 128 + 16 * c)  # block11 lower cols
                    a_t = a_pool.tile([128, W, D], F16, name="a_t", tag="a_t")
                    b_t = b_pool.tile([128, W, D], F16, name="b_t", tag="b_t")
                    nc.sync.dma_start(
                        a_t[0:p, :, :], depA[0:p, cols0[0] : cols0[1], :]
                    )
                    nc.sync.dma_start(
                        a_t[p:128, :, :],
                        depA[128 + p : 256, cols1[0] : cols1[1], :],
                    )
                    nc.scalar.dma_start(
                        b_t[0:p, :, :], depB[0:p, cols0[0] : cols0[1], :]
                    )
                    nc.scalar.dma_start(
                        b_t[p:128, :, :],
                        depB[128 + p : 256, cols1[0] : cols1[1], :],
                    )
                    chain_and_reduce(
                        a_t,
                        b_t,
                        [
                            (score[0][0:p, cols0[0] : cols0[1]], 0, p),
                            (score[1][p:128, cols1[0] : cols1[1]], p, 128),
                        ],
                    )

                # --- off-diagonal block (0,1), full, chunked ---
                for c in range(NCH):
                    j0 = 128 + 16 * c
                    a_t = a_pool.tile([128, W, D], F16, name="a_t", tag="a_t")
                    b_t = b_pool.tile([128, W, D], F16, name="b_t", tag="b_t")
                    nc.sync.dma_start(a_t[:], depA[0:128, j0 : j0 + 16, :])
                    nc.scalar.dma_start(b_t[:], depB[0:128, j0 : j0 + 16, :])
                    chain_and_reduce(a_t, b_t, [(score[0][:, j0 : j0 + 16], 0, 128)])

                # --- mirrors for the diagonal blocks (dep-only) ---
                psum_t = [
                    psp.tile([128, 128], F32, tag=f"pt{i}", name=f"pt{i}", bufs=1)
                    for i in range(2)
                ]
                nc.tensor.transpose(psum_t[0][:], score[0][:, 0:128], id_t[:])
                nc.vector.tensor_add(
                    score[0][:, 0:128], score[0][:, 0:128], psum_t[0][:]
                )
                nc.tensor.transpose(psum_t[1][:], score[1][:, 128:256], id_t[:])
                nc.vector.tensor_add(
                    score[1][:, 128:256], score[1][:, 128:256], psum_t[1][:]
                )
                # scatter diagonal minis (after mirror: overwrites zeros)
                for blk in range(2):
                    base = 128 * blk
                    for m in range(NCH):
                        lo, hi = 16 * m, 16 * m + 16
                        nc.scalar.copy(
                            score[blk][lo:hi, base + lo : base + hi],
                            mtmp[blk][lo:hi, :],
                        )

                # --- val adds ---
                nc.vector.tensor_add(
                    score[0][:, 0:128], score[0][:, 0:128], psum_sv[0][:, 0:128]
                )
                nc.vector.tensor_add(
                    score[0][:, 128:256],
                    score[0][:, 128:256],
                    psum_sv[0][:, 128:256],
                )
                nc.vector.tensor_add(
                    score[1][:, 128:256],
                    score[1][:, 128:256],
                    psum_sv[1][:, 128:256],
                )
                # (1,0) = transpose of full (0,1) incl. val (both symmetric)
                psum_t2 = psp.tile([128, 128], F32, tag="pt2", name="pt2", bufs=1)
                nc.tensor.transpose(psum_t2[:], score[0][:, 128:256], id_t[:])
                nc.scalar.copy(score[1][:, 0:128], psum_t2[:])

                # --- epilogue, split by halves for tail overlap ---
                for i in range(2):
                    for k, (lo, hi) in enumerate(((0, 128), (128, 256))):
                        nc.scalar.activation(
                            expv[i][:, lo:hi],
                            score[i][:, lo:hi],
                            mybir.ActivationFunctionType.Exp,
                            scale=float(SCALE),
                        )
                        nc.vector.tensor_mul(
                            expv[i][:, lo:hi], expv[i][:, lo:hi], adj_t[i][:, lo:hi]
                        )
                        nc.vector.reduce_sum(
                            (den[i] if k == 0 else denb[i])[:],
                            expv[i][:, lo:hi],
                            axis=mybir.AxisListType.X,
                        )
                    nc.vector.tensor_add(den[i][:], den[i][:], denb[i][:])
                    nc.vector.tensor_scalar_add(den[i][:], den[i][:], float(EPS))
                    nc.vector.reciprocal(rec[i][:], den[i][:])
                    nc.vector.tensor_scalar_mul(expv[i][:], expv[i][:], rec[i][:, 0:1])
                    nc.sync.dma_start(out[128 * i : 128 * (i + 1), :], expv[i][:])

    nc.compile()
    return nc


def make_in_map(val_out_b, dep_b, adj_b):
    """Per-core input dict from one batch element's full-precision inputs."""
    return {
        "depA": np.ascontiguousarray(dep_b).astype(np.float16),
        "depB": np.ascontiguousarray(dep_b.transpose(1, 0, 2)).astype(np.float16),
        "valT": np.ascontiguousarray(val_out_b.T).astype(np.float32),
        "adj": np.ascontiguousarray(adj_b).astype(np.float32),
        "ident": np.eye(128, dtype=np.float32),
    }


def _get_nc():
    global _NC
    if _NC is None:
        _NC = build_nc()
    return _NC


def kernel(val_out, dep_embed, adj):
    val_out = np.asarray(val_out, dtype=np.float32)
    dep_embed = np.asarray(dep_embed, dtype=np.float32)
    adj = np.asarray(adj, dtype=np.float32)
    assert val_out.shape == (B, N, D)
    assert dep_embed.shape == (B, N, N, D)
    assert adj.shape == (B, N, N)

    nc = _get_nc()
    in_maps = [make_in_map(val_out[b], dep_embed[b], adj[b]) for b in range(B)]
    res = run_bass_kernel_spmd(nc, in_maps, core_ids=list(range(B)))
    return np.stack([r["out"] for r in res.results])


# revision 6
# speedup vs baseline: 4.9056x; 4.9056x over previous
"""DepAttention kernel for Trainium2 (Bass/Tile), data-parallel over batch.

score[b,i,j] = (<val[b,i],val[b,j]> + <dep[b,i,j],dep[b,j,i]>) / sqrt(D)
out = exp(score)*adj / (rowsum(exp(score)*adj) + 1e-10)

score is symmetric in (i,j), so each core (one batch element) computes
each unordered pair's dep dot product exactly once and mirrors the rest
with PE transposes. dep is sent in TWO fp16 copies chosen so every DMA
is contiguous 4KB-per-partition: depA[i,j,:] = dep[i,j,:] and
depB[i,j,:] = dep[j,i,:]; the product depA*depB followed by a sum over
d gives the dep score. fp16 halves both HBM traffic and DVE multiply
time (2x packed mode); the d-reduction runs as an in-place fp16 add
tree (128->64->32->16, each step 2x) and a final f32 reduce_sum over 16.

Work partition per 128-row block pair (N=256 = 2 blocks):
 - off-diag block (0,1): full 128x128, chunked by 16 cols; mirrored
   into (1,0) by transposing after the val part is added (val score is
   symmetric too).
 - diagonal blocks: strict-triangle column chunks packed EXACTLY into
   128 partitions: block00's upper chunk c (partitions 0..16c) rides
   with block11's lower chunk c-1 (partitions 16c..128) in one tile,
   c = 1..7. Both reduce straight into their score tiles at native
   partitions. The 16x16 diagonal mini-squares are computed full
   (both triangles at once) as one stacked [128,16,128] tile per block
   and scattered after the mirror-add, so no masks are needed: at
   mirror time the uncomputed regions are zero.
"""

import numpy as np

import concourse.bacc as bacc
import concourse.tile as tile
import concourse.mybir as mybir
from concourse.bass_utils import run_bass_kernel_spmd

B, N, D = 8, 256, 128
W = 32        # columns per chunk
NCH = 128 // W
SCALE = 1.0 / np.sqrt(np.float32(D))
EPS = 1e-10
F32 = mybir.dt.float32
F16 = mybir.dt.float16

_NC = None


def build_nc(reps=1):
    """reps>1 unrolls the whole computation (for paired-slope timing)."""
    nc = bacc.Bacc("TRN2", target_bir_lowering=False, debug=False, num_devices=8)

    depA = nc.dram_tensor("depA", [N, N, D], F16, kind="ExternalInput")
    depB = nc.dram_tensor("depB", [N, N, D], F16, kind="ExternalInput")
    valT = nc.dram_tensor("valT", [D, N], F32, kind="ExternalInput")
    adj = nc.dram_tensor("adj", [N, N], F32, kind="ExternalInput")
    ident = nc.dram_tensor("ident", [128, 128], F32, kind="ExternalInput")
    out = nc.dram_tensor("out", [N, N], F32, kind="ExternalOutput")

    with tile.TileContext(nc) as tc:
        with (
            tc.tile_pool(name="a", bufs=4) as a_pool,
            tc.tile_pool(name="b", bufs=4) as b_pool,
            tc.tile_pool(name="persist", bufs=1) as pp,
            tc.tile_pool(name="psum", bufs=1, space="PSUM") as psp,
        ):
            # persistent tiles
            vt = pp.tile([D, N], F32, tag="vt")
            id_t = pp.tile([128, 128], F32, tag="id")
            adj_t = [
                pp.tile([128, N], F32, tag=f"adj{i}", name=f"adj{i}") for i in range(2)
            ]
            scratch = pp.tile([128, 1], F32, tag="scratch")

            nc.gpsimd.dma_start(vt[:], valT[:])
            nc.gpsimd.dma_start(id_t[:], ident[:])
            for i in range(2):
                nc.gpsimd.dma_start(adj_t[i][:], adj[128 * i : 128 * (i + 1), :])
            # prime the ACT exp table before the epilogue needs it
            nc.vector.memset(scratch[:], 0.0)
            nc.scalar.activation(
                scratch[:], scratch[:], mybir.ActivationFunctionType.Exp, scale=1.0
            )

            def pwin(lo, hi):
                """Split [lo,hi) into HW-legal partition windows
                (starts 0/32/64/96; span 64 only from 0/64, 128 from 0)."""
                res = []
                while lo < hi:
                    if lo == 0 and hi >= 128:
                        w = 128
                    elif lo % 64 == 0 and hi - lo >= 64:
                        w = 64
                    else:
                        w = min(32, hi - lo)
                    res.append((lo, lo + w))
                    lo += w
                return res

            def chain_and_reduce(a_t, b_t, outs):
                """In-place fp16 add tree over d then f32 reduces.
                outs = list of (score_tile, col_lo, col_hi, part_lo, part_hi)."""
                nc.vector.tensor_mul(a_t[:], a_t[:], b_t[:])
                nc.vector.tensor_add(
                    a_t[:, :, 0:64], a_t[:, :, 0:64], a_t[:, :, 64:128]
                )
                nc.vector.tensor_add(
                    a_t[:, :, 0:32], a_t[:, :, 0:32], a_t[:, :, 32:64]
                )
                nc.vector.tensor_add(
                    a_t[:, :, 0:16], a_t[:, :, 0:16], a_t[:, :, 16:32]
                )
                for st, clo, chi, plo, phi in outs:
                    for wlo, whi in pwin(plo, phi):
                        nc.vector.reduce_sum(
                            st[wlo:whi, clo:chi],
                            a_t[wlo:whi, :, 0:16],
                            axis=mybir.AxisListType.X,
                        )

            for _rep in range(reps):
                score = [
                    pp.tile([128, N], F32, tag=f"score{i}", name=f"score{i}", bufs=2)
                    for i in range(2)
                ]
                expv = [
                    pp.tile([128, N], F32, tag=f"expv{i}", name=f"expv{i}", bufs=2)
                    for i in range(2)
                ]
                mtmp = [
                    pp.tile([128, W], F32, tag=f"mtmp{i}", name=f"mtmp{i}", bufs=2)
                    for i in range(2)
                ]
                den = [
                    pp.tile([128, 1], F32, tag=f"den{i}", name=f"den{i}", bufs=2)
                    for i in range(2)
                ]
                denb = [
                    pp.tile([128, 1], F32, tag=f"denb{i}", name=f"denb{i}", bufs=2)
                    for i in range(2)
                ]
                rec = [
                    pp.tile([128, 1], F32, tag=f"rec{i}", name=f"rec{i}", bufs=2)
                    for i in range(2)
                ]
                psum_sv = [
                    psp.tile([128, N], F32, tag=f"sv{i}", name=f"sv{i}", bufs=1)
                    for i in range(2)
                ]

                # zero regions that the mirror-add will read where uncomputed
                nc.gpsimd.memset(score[0][:, 0:128], 0.0)
                nc.gpsimd.memset(score[1][:, 128:256], 0.0)

                # val part: score_val[I] = valT[:, I*128:+128].T @ valT
                for i in range(2):
                    nc.tensor.matmul(
                        psum_sv[i][:],
                        vt[:, 128 * i : 128 * (i + 1)],
                        vt[:],
                        start=True,
                        stop=True,
                    )

                # --- diagonal 16x16 mini-squares, stacked per block ---
                for blk in range(2):
                    base = 128 * blk
                    a_t = a_pool.tile([128, W, D], F16, name="a_t", tag="a_t")
                    b_t = b_pool.tile([128, W, D], F16, name="b_t", tag="b_t")
                    for m in range(NCH):
                        lo, hi = W * m, W * m + W
                        nc.sync.dma_start(
                            a_t[lo:hi, :, :],
                            depA[base + lo : base + hi, base + lo : base + hi, :],
                        )
                        nc.scalar.dma_start(
                            b_t[lo:hi, :, :],
                            depB[base + lo : base + hi, base + lo : base + hi, :],
                        )
                    chain_and_reduce(a_t, b_t, [(mtmp[blk], 0, W, 0, 128)])

                # --- diagonal strict-triangle pair chunks c=1..7 ---
                for c in range(1, NCH):
                    p = W * c
                    cols0 = (W * c, W * c + W)          # block00 upper cols
                    cols1 = (128 + W * (c - 1), 128 + W * c)  # block11 lower cols
                    a_t = a_pool.tile([128, W, D], F16, name="a_t", tag="a_t")
                    b_t = b_pool.tile([128, W, D], F16, name="b_t", tag="b_t")
                    nc.sync.dma_start(
                        a_t[0:p, :, :], depA[0:p, cols0[0] : cols0[1], :]
                    )
                    nc.sync.dma_start(
                        a_t[p:128, :, :],
                        depA[128 + p : 256, cols1[0] : cols1[1], :],
                    )
                    nc.scalar.dma_start(
                        b_t[0:p, :, :], depB[0:p, cols0[0] : cols0[1], :]
                    )
                    nc.scalar.dma_start(
                        b_t[p:128, :, :],
                        depB[128 + p : 256, cols1[0] : cols1[1], :],
                    )
                    chain_and_reduce(
                        a_t,
                        b_t,
                        [
                            (score[0], cols0[0], cols0[1], 0, p),
                            (score[1], cols1[0], cols1[1], p, 128),
                        ],
                    )

                # --- off-diagonal block (0,1), full, chunked ---
                for c in range(NCH):
                    j0 = 128 + W * c
                    a_t = a_pool.tile([128, W, D], F16, name="a_t", tag="a_t")
                    b_t = b_pool.tile([128, W, D], F16, name="b_t", tag="b_t")
                    nc.sync.dma_start(a_t[:], depA[0:128, j0 : j0 + W, :])
                    nc.scalar.dma_start(b_t[:], depB[0:128, j0 : j0 + W, :])
                    chain_and_reduce(a_t, b_t, [(score[0], j0, j0 + W, 0, 128)])

                # --- mirrors for the diagonal blocks (dep-only) ---
                psum_t = [
                    psp.tile([128, 128], F32, tag=f"pt{i}", name=f"pt{i}", bufs=1)
                    for i in range(2)
                ]
                nc.tensor.transpose(psum_t[0][:], score[0][:, 0:128], id_t[:])
                nc.vector.tensor_add(
                    score[0][:, 0:128], score[0][:, 0:128], psum_t[0][:]
                )
                nc.tensor.transpose(psum_t[1][:], score[1][:, 128:256], id_t[:])
                nc.vector.tensor_add(
                    score[1][:, 128:256], score[1][:, 128:256], psum_t[1][:]
                )
                # scatter diagonal minis (after mirror: overwrites zeros)
                for blk in range(2):
                    base = 128 * blk
                    for m in range(NCH):
                        lo, hi = W * m, W * m + W
                        nc.scalar.copy(
                            score[blk][lo:hi, base + lo : base + hi],
                            mtmp[blk][lo:hi, :],
                        )

                # --- val adds ---
                nc.vector.tensor_add(
                    score[0][:, 0:128], score[0][:, 0:128], psum_sv[0][:, 0:128]
                )
                nc.vector.tensor_add(
                    score[0][:, 128:256],
                    score[0][:, 128:256],
                    psum_sv[0][:, 128:256],
                )
                nc.vector.tensor_add(
                    score[1][:, 128:256],
                    score[1][:, 128:256],
                    psum_sv[1][:, 128:256],
                )
                # (1,0) = transpose of full (0,1) incl. val (both symmetric)
                psum_t2 = psp.tile([128, 128], F32, tag="pt2", name="pt2", bufs=1)
                nc.tensor.transpose(psum_t2[:], score[0][:, 128:256], id_t[:])
                nc.scalar.copy(score[1][:, 0:128], psum_t2[:])

                # --- epilogue, split by halves for tail overlap ---
                for i in range(2):
                    for k, (lo, hi) in enumerate(((0, 128), (128, 256))):
                        nc.scalar.activation(
                            expv[i][:, lo:hi],
                            score[i][:, lo:hi],
                            mybir.ActivationFunctionType.Exp,
                            scale=float(SCALE),
                        )
                        nc.vector.tensor_mul(
                            expv[i][:, lo:hi], expv[i][:, lo:hi], adj_t[i][:, lo:hi]
                        )
                        nc.vector.reduce_sum(
                            (den[i] if k == 0 else denb[i])[:],
                            expv[i][:, lo:hi],
                            axis=mybir.AxisListType.X,
                        )
                    nc.vector.tensor_add(den[i][:], den[i][:], denb[i][:])
                    nc.vector.tensor_scalar_add(den[i][:], den[i][:], float(EPS))
                    nc.vector.reciprocal(rec[i][:], den[i][:])
                    nc.vector.tensor_scalar_mul(expv[i][:], expv[i][:], rec[i][:, 0:1])
                    nc.sync.dma_start(out[128 * i : 128 * (i + 1), :], expv[i][:])

    nc.compile()
    return nc


def make_in_map(val_out_b, dep_b, adj_b):
    """Per-core input dict from one batch element's full-precision inputs."""
    return {
        "depA": np.ascontiguousarray(dep_b).astype(np.float16),
        "depB": np.ascontiguousarray(dep_b.transpose(1, 0, 2)).astype(np.float16),
        "valT": np.ascontiguousarray(val_out_b.T).astype(np.float32),
        "adj": np.ascontiguousarray(adj_b).astype(np.float32),
        "ident": np.eye(128, dtype=np.float32),
    }


def _get_nc():
    global _NC
    if _NC is None:
        _NC = build_nc()
    return _NC


def kernel(val_out, dep_embed, adj):
    val_out = np.asarray(val_out, dtype=np.float32)
    dep_embed = np.asarray(dep_embed, dtype=np.float32)
    adj = np.asarray(adj, dtype=np.float32)
    assert val_out.shape == (B, N, D)
    assert dep_embed.shape == (B, N, N, D)
    assert adj.shape == (B, N, N)

    nc = _get_nc()
    in_maps = [make_in_map(val_out[b], dep_embed[b], adj[b]) for b in range(B)]
    res = run_bass_kernel_spmd(nc, in_maps, core_ids=list(range(B)))
    return np.stack([r["out"] for r in res.results])
